# revision 21
# baseline (speedup 1.0000x reference)
"""Trainium2 Bass kernel for nn_InceptionTraversal (hierarchical sphere-softmax
MoE routing + per-band sigmoid routers).

Strategy (v3)
-------------
Host (numpy):
  * All distances d_s = |M_s p + u_s|^2 for the 84 spheres (4 + 16 + 64, with
    portal affines composed) are linear in psi = [x2,xy,xz,y2,yz,z2,x,y,z,1].
    Fold alpha = 1/(2T^2+eps) into Wd [10, 84].
  * Leaf-fold: d123[jlm] = d1[j] + d2[jl] + d3[jlm] is ALSO linear in psi ->
    extra 64 matmul columns, so the device needs no level-combine adds for
    the softmax numerators (Z1 cancels in the final normalize; raw d1 is
    never needed in the main phase).
  * Per-band routers are one block-diag matmul [64 -> 256] with sigmoid(x)
    = 0.5 + 0.5*tanh(x/2) folded into the weights.
  * Ship Phi = [psi(10); spectral(64)] bf16, pre-chunked [NSC, 74, CH].
Device (per core, 16384 tokens = 32 superchunks of 4x128-token groups):
  Phase A (sqrt table): bf16 matmul d-only [10 -> 84]; u = lam*sqrt(d+eps);
    DVE folds U123 = u1 + u2 + u3 into a [64/token] SBUF stash.
  Phase B (exp/tanh table; no barrier, ACT program order gates the switch):
    bf16 matmul [74 -> 400] = [router(256) | d23(80) | dfold(64)] per group.
    T3 = dfold + U123 added in-place in PSUM, then ONE fused exp over
    [d23|T3] gives E (denominator terms) and H (leaf numerators) in one
    ACT op; th = tanh(router).  Tail (Z3/Z2 reduces, R = 1/(Z2*Z3), m3e,
    band-sum, (sth+4)*m3e, row-normalize) is batched over superchunk PAIRS
    to halve DVE/GPSIMD instruction overhead; output written bf16.
Sharding: pure data-parallel over 8 cores (tokens split 8 ways).
"""

import sys

import numpy as np

if "/opt/trn_rl_repo" not in sys.path:
    sys.path.insert(0, "/opt/trn_rl_repo")

# ---- problem constants (hardcoded per contest contract) ----
N_DOM, N_SUB, N_CON = 4, 4, 4
SPECTRAL_DIM, N_BANDS = 64, 4
BAND_SIZE = SPECTRAL_DIM // N_BANDS
TEMP, LAM, EPS = 1.0, 0.1, 1e-8
ALPHA = 1.0 / (2.0 * TEMP * TEMP + EPS)
N_CORES = 8
B, S = 16, 8192
NTOK = B * S
TPC = NTOK // N_CORES          # tokens per core = 16384
GRP = 128                      # tokens per matmul group
G = 4                          # groups per superchunk (PSUM ping-pong)
CH = G * GRP                   # 512 tokens per superchunk
NSC = TPC // CH                # superchunks = 32
NS = 84                        # spheres (4 + 16 + 64)
NR = 256                       # router logits (64 leaves x 4 bands, (k,n))
NL = 64                        # leaves
KF = 74                        # Phi rows: 9 psi + 1 ones + 64 spectral
KD = 10                        # rows used by the distance matmul
# phase-B matmul column layout: [router 0:256 | d23 256:336 | dfold 336:400]
C_R0, C_D0, C_F0, NCOL = 0, NR, NR + 80, NR + 80 + NL

_compiled = {}


def _host_matrices(centers1, centers2, centers3, portal1_T, portal2_T,
                   W_bands, b_bands, band_weights):
    """Build Wd [10,84] (phase A), Wc [74,400] (phase B). float64 inside."""
    c1 = centers1.astype(np.float64)
    c2 = centers2.astype(np.float64)
    c3 = centers3.astype(np.float64)
    A1 = portal1_T[:, :, :3].astype(np.float64)
    b1 = portal1_T[:, :, 3].astype(np.float64)
    A2 = portal2_T[:, :, :3].astype(np.float64)
    b2 = portal2_T[:, :, 3].astype(np.float64)

    Ms = np.zeros((NS, 3, 3))
    us = np.zeros((NS, 3))
    s = 0
    for j in range(N_DOM):                     # level 1
        Ms[s] = np.eye(3)
        us[s] = -c1[j]
        s += 1
    for j in range(N_DOM):                     # level 2
        for l in range(N_SUB):
            Ms[s] = A1[j]
            us[s] = b1[j] - c2[j * N_SUB + l]
            s += 1
    for j in range(N_DOM):                     # level 3
        for l in range(N_SUB):
            jl = j * N_SUB + l
            M = A2[jl] @ A1[j]
            v = A2[jl] @ b1[j] + b2[jl]
            for m in range(N_CON):
                Ms[s] = M
                us[s] = v - c3[jl * N_CON + m]
                s += 1
    assert s == NS

    Wd = np.zeros((KD, NS))
    for i in range(NS):
        Q = Ms[i].T @ Ms[i]
        lin = 2.0 * (Ms[i].T @ us[i])
        Wd[:, i] = [Q[0, 0], 2 * Q[0, 1], 2 * Q[0, 2], Q[1, 1], 2 * Q[1, 2],
                    Q[2, 2], lin[0], lin[1], lin[2], us[i] @ us[i]]
    Wd *= ALPHA                                # PSUM d-cols = alpha * d_true

    w = np.exp(band_weights.astype(np.float64))
    w = w / w.sum()
    equal_w = bool(np.allclose(w, w[0], rtol=1e-6, atol=1e-9))

    Wc = np.zeros((KF, NCOL))
    # router cols: col k*4 + n = 0.5 * (x_band_n . W_bands[n,:,k] + b[n,k])
    Wr = np.zeros((SPECTRAL_DIM, SPECTRAL_DIM, N_BANDS))
    for n in range(N_BANDS):
        Wr[n * BAND_SIZE:(n + 1) * BAND_SIZE, :, n] = 0.5 * W_bands[n].astype(np.float64)
    Wc[KD:KF, C_R0:C_R0 + NR] = Wr.reshape(SPECTRAL_DIM, NR)
    Wc[KD - 1, C_R0:C_R0 + NR] = 0.5 * b_bands.astype(np.float64).T.reshape(NR)
    # raw d for levels 2+3 (E / Z denominators)
    Wc[0:KD, C_D0:C_D0 + 80] = Wd[:, 4:84]
    # leaf-folded d123 = d3 + d2[parent] + d1[grandparent]
    dfold = (Wd[:, 20:84]
             + np.repeat(Wd[:, 4:20], N_CON, axis=1)
             + np.repeat(Wd[:, 0:4], N_SUB * N_CON, axis=1))
    Wc[0:KD, C_F0:C_F0 + NL] = dfold
    return (Wd.astype(np.float32), Wc.astype(np.float32), equal_w,
            w.astype(np.float32))


def _host_phi(pos_3d, spectral_color):
    """Phi [74, NTOK] f32: rows [x2,xy,xz,y2,yz,z2,x,y,z,1, spectral...]."""
    p = pos_3d.reshape(-1, 3).astype(np.float32)
    x, y, z = p[:, 0], p[:, 1], p[:, 2]
    phi = np.empty((KF, NTOK), dtype=np.float32)
    phi[0] = x * x
    phi[1] = x * y
    phi[2] = x * z
    phi[3] = y * y
    phi[4] = y * z
    phi[5] = z * z
    phi[6] = x
    phi[7] = y
    phi[8] = z
    phi[9] = 1.0
    phi[KD:] = spectral_color.reshape(-1, SPECTRAL_DIM).astype(np.float32).T
    return np.ascontiguousarray(phi)


def _build_module(equal_w):
    import concourse.bacc as bacc
    import concourse.mybir as mybir
    import concourse.tile as tile

    f32 = mybir.dt.float32
    f16 = mybir.dt.float16
    bf16 = mybir.dt.bfloat16
    AF = mybir.ActivationFunctionType
    OP = mybir.AluOpType
    AX = mybir.AxisListType

    nc = bacc.Bacc("TRN2", target_bir_lowering=False)
    phi_d = nc.dram_tensor("phi", [NSC * KF, CH], f16, kind="ExternalInput")
    wd_d = nc.dram_tensor("wd", [KD, NS], f16, kind="ExternalInput")
    wc_d = nc.dram_tensor("wc", [KF, NCOL], f16, kind="ExternalInput")
    out_d = nc.dram_tensor("routing", [NSC * GRP, G * NL], f32,
                           kind="ExternalOutput")

    # numeric folds:  u = sqrt(sq_scale*dps + sq_bias) = lam*sqrt(d_true+eps)
    sq_scale = (LAM * LAM) / ALPHA
    sq_bias = LAM * LAM * EPS
    pre_add = 4.0 if equal_w else 1.0

    # activation() turns float biases into const APs - register ours.
    for cval in (sq_bias,):
        if (f32, cval) not in nc.const_aps.aps:
            ct = nc.alloc_sbuf_tensor(f"const-f32-{cval}", [128, 1], f32)
            nc.gpsimd.memset(ct.ap(), cval)
            nc.const_aps.aps[(f32, cval)] = ct.ap()
    nc.all_engine_barrier()

    NP = NSC // 2                      # superchunk pairs

    with tile.TileContext(nc) as tc:
        with (
            tc.tile_pool(name="const", bufs=1) as constp,
            tc.tile_pool(name="stash", bufs=1) as stashp,
            tc.tile_pool(name="io", bufs=3) as iop,
            tc.tile_pool(name="work", bufs=3) as wp,
            tc.tile_pool(name="tail", bufs=2) as tp,
            tc.tile_pool(name="ps", bufs=2, space="PSUM") as psp,
        ):
            wd_sb = constp.tile([KD, NS], f16)
            nc.sync.dma_start(wd_sb[:], wd_d[:])
            wc_sb = constp.tile([KF, NCOL], f16)
            nc.sync.dma_start(wc_sb[:], wc_d[:])
            if not equal_w:
                wt_sb = constp.tile([GRP, N_BANDS], f32)
                wt_dram = nc.dram_tensor("wt", [1, N_BANDS], f32,
                                         kind="ExternalInput")
                nc.sync.dma_start(wt_sb[:], wt_dram[:].partition_broadcast(GRP))

            u_stash = stashp.tile([GRP, NSC * G * NL], f16)

            # ---------------- Phase A: sqrt table set ----------------
            for pp in range(NP):
                u = wp.tile([GRP, 2, G, NS], f16, tag="u")
                for h in range(2):
                    sc = 2 * pp + h
                    psiA = iop.tile([KD, CH], f16, tag="psiA")
                    nc.sync.dma_start(psiA[:], phi_d[sc * KF:sc * KF + KD, :])
                    psA = psp.tile([GRP, G, 512], f32, tag="ps")
                    for g in range(G):
                        nc.tensor.matmul(
                            psA[:, g, 0:NS],
                            psiA[:, g * GRP:(g + 1) * GRP],
                            wd_sb[:],
                            start=True, stop=True,
                        )
                    nc.scalar.activation(u[:, h], psA[:, :, 0:NS],
                                         AF.Sqrt, bias=sq_bias, scale=sq_scale)
                U12 = wp.tile([GRP, 2, G, 16], f16, tag="U12")
                nc.vector.tensor_tensor(
                    U12.rearrange("p h g (j l) -> p h g j l", l=4),
                    u[:, :, :, 4:20].rearrange("p h g (j l) -> p h g j l", l=4),
                    u[:, :, :, 0:4].unsqueeze(4).broadcast_to((GRP, 2, G, 4, 4)),
                    OP.add)
                ust = u_stash[:, pp * (2 * G * NL):(pp + 1) * (2 * G * NL)]
                nc.vector.tensor_tensor(
                    ust.rearrange("p (h g jl m) -> p h g jl m", h=2, g=G, m=4),
                    u[:, :, :, 20:NS].rearrange("p h g (jl m) -> p h g jl m", m=4),
                    U12[:].unsqueeze(4).broadcast_to((GRP, 2, G, 16, 4)),
                    OP.add)

            # prefetch the first pair's phi chunks so the transfers run
            # during the barrier drain
            pf_tiles = []
            for sc in range(2):
                phiB = iop.tile([KF, CH], f16, tag="phiB")
                eng = nc.gpsimd if (sc % 2 == 0) else nc.sync
                eng.dma_start(phiB[:], phi_d[sc * KF:(sc + 1) * KF, :])
                pf_tiles.append(phiB)

            tc.strict_bb_all_engine_barrier()

            # ---------------- Phase B: exp/tanh table set ----------------
            # No engine barrier: the ACT sequencer runs all sqrts, then the
            # auto-inserted table switch, then the exps; other engines run
            # ahead freely.  Tail math is batched over superchunk pairs.
            for pp in range(NP):
                # X = exp(-U123): ray-decay leaf factor; depends only on the
                # phase-A stash, so emit it first to unblock the GP chain
                X = tp.tile([GRP, 2, G, NL], bf16, tag="X")
                ustp = u_stash[:, pp * (2 * G * NL):(pp + 1) * (2 * G * NL)]
                nc.scalar.activation(
                    X[:], ustp.rearrange("p (h g c) -> p h g c", h=2, g=G),
                    AF.Exp, scale=-1.0)

                EH = tp.tile([GRP, 2, G, 144], bf16, tag="EH")
                th = tp.tile([GRP, 2, G, NR], f16, tag="th")
                for h in range(2):
                    sc = 2 * pp + h
                    if pp == 0:
                        phiB = pf_tiles[h]
                    else:
                        phiB = iop.tile([KF, CH], f16, tag="phiB")
                        eng = nc.gpsimd if (sc % 2 == 0) else nc.sync
                        eng.dma_start(phiB[:], phi_d[sc * KF:(sc + 1) * KF, :])
                    psB = psp.tile([GRP, G, 512], f32, tag="ps")
                    for g in range(G):
                        nc.tensor.matmul(
                            psB[:, g, 0:NCOL],
                            phiB[:, g * GRP:(g + 1) * GRP],
                            wc_sb[:],
                            start=True, stop=True,
                        )
                    # one fused exp: E = exp(-d23) | Hf = exp(-dfold)
                    nc.scalar.activation(EH[:, h], psB[:, :, C_D0:C_D0 + 144],
                                         AF.Exp, scale=-1.0)
                    nc.scalar.activation(th[:, h], psB[:, :, C_R0:C_R0 + NR],
                                         AF.Tanh)

                E = EH[:, :, :, 0:80]
                Hf = EH[:, :, :, 80:144]
                # merged Z2|Z3 denominator sums: [p,h,g,20]
                Z23 = tp.tile([GRP, 2, G, 20], f32, tag="Z23")
                nc.vector.tensor_reduce(
                    Z23[:], E.rearrange("p h g (q m) -> p h g q m", m=4),
                    AX.X, OP.add)
                ZZ = tp.tile([GRP, 2, G, 16], f32, tag="ZZ")
                nc.gpsimd.tensor_tensor(
                    ZZ.rearrange("p h g (j l) -> p h g j l", l=4),
                    Z23[:, :, :, 4:20].rearrange("p h g (j l) -> p h g j l", l=4),
                    Z23[:, :, :, 0:4].unsqueeze(4).broadcast_to((GRP, 2, G, 4, 4)),
                    OP.mult)
                R = tp.tile([GRP, 2, G, 16], f32, tag="R")
                nc.vector.reciprocal_approx_fast(
                    R[:].rearrange("p h g c -> p (h g c)"),
                    ZZ[:].rearrange("p h g c -> p (h g c)"))

                # H = Hf * X (leaf numerator incl ray decay), all-f16 on DVE
                HX = tp.tile([GRP, 2, G, NL], bf16, tag="HX")
                nc.gpsimd.tensor_tensor(HX[:], Hf, X[:], OP.mult)
                m3e = tp.tile([GRP, 2, G, NL], f32, tag="m3e")
                nc.vector.tensor_tensor(
                    m3e.rearrange("p h g (jl m) -> p h g jl m", m=4),
                    HX[:].rearrange("p h g (jl m) -> p h g jl m", m=4),
                    R[:].unsqueeze(4).broadcast_to((GRP, 2, G, 16, 4)),
                    OP.mult)

                # band sum of tanh in two f16 steps (2x DVE mode on step 1)
                thv = th[:].rearrange("p h g (k n) -> p h g k n", n=4)
                if equal_w:
                    s2 = tp.tile([GRP, 2, G, NL, 2], f16, tag="s2")
                    nc.vector.tensor_tensor(
                        s2[:], thv[:, :, :, :, 0:2], thv[:, :, :, :, 2:4],
                        OP.add)
                else:
                    thw = tp.tile([GRP, 2, G, NR], f16, tag="thw")
                    nc.vector.tensor_tensor(
                        thw.rearrange("p h g (k n) -> p h g k n", n=4), thv,
                        wt_sb[:].unsqueeze(1).unsqueeze(1).unsqueeze(1)
                        .broadcast_to((GRP, 2, G, NL, N_BANDS)),
                        OP.mult)
                    s2 = tp.tile([GRP, 2, G, NL, 2], f16, tag="s2")
                    thwv = thw.rearrange("p h g (k n) -> p h g k n", n=4)
                    nc.vector.tensor_tensor(
                        s2[:], thwv[:, :, :, :, 0:2], thwv[:, :, :, :, 2:4],
                        OP.add)
                sth = tp.tile([GRP, 2, G, NL], f32, tag="sth")
                nc.vector.tensor_tensor(
                    sth[:], s2[:, :, :, :, 0], s2[:, :, :, :, 1], OP.add)

                # unnormalized pre is stored; the host does the row-
                # normalize (single divide by the row sum)
                pre = tp.tile([GRP, 2, G, NL], f32, tag="pre")
                nc.vector.scalar_tensor_tensor(
                    pre[:], sth[:], pre_add, m3e[:], OP.add, OP.mult)

                nc.sync.dma_start(
                    out_d[pp * 2 * GRP:(pp + 1) * 2 * GRP, :].rearrange(
                        "(h p) (g k) -> p h g k", p=GRP, g=G),
                    pre[:])

    nc.finalize()
    return nc


def _get_compiled(equal_w):
    if equal_w not in _compiled:
        _compiled[equal_w] = _build_module(equal_w)
    return _compiled[equal_w]


def _chunk_rows(a, nrows):
    """[nrows, TPC] f32 -> [NSC*nrows, CH] fp16 (contiguous per superchunk)."""
    v = a.reshape(nrows, NSC, CH).transpose(1, 0, 2)
    return np.ascontiguousarray(v.reshape(NSC * nrows, CH)).astype(np.float16)


def _build_inmaps(pos_3d, spectral_color, centers1, centers2, centers3,
                  portal1_T, portal2_T, W_bands, b_bands, band_weights):
    Wd, Wc, equal_w, w_vec = _host_matrices(
        np.asarray(centers1), np.asarray(centers2), np.asarray(centers3),
        np.asarray(portal1_T), np.asarray(portal2_T),
        np.asarray(W_bands), np.asarray(b_bands), np.asarray(band_weights))
    phi = _host_phi(np.asarray(pos_3d), np.asarray(spectral_color))

    wd_b = Wd.astype(np.float16)
    wc_b = Wc.astype(np.float16)
    in_maps = []
    for c in range(N_CORES):
        pc = phi[:, c * TPC:(c + 1) * TPC]
        m = {
            "phi": _chunk_rows(pc, KF),
            "wd": wd_b,
            "wc": wc_b,
        }
        if not equal_w:
            m["wt"] = w_vec.reshape(1, N_BANDS)
        in_maps.append(m)
    return in_maps, equal_w


def _unshard_out(res):
    outs = []
    for c in range(N_CORES):
        o = np.asarray(res.results[c]["routing"]).astype(np.float32)
        o = o.reshape(NSC, GRP, G, NL).transpose(0, 2, 1, 3).reshape(TPC, NL)
        outs.append(o)
    pre = np.concatenate(outs, axis=0)
    # final row-normalize (matches reference's clip(sum, 1e-8))
    pre /= np.maximum(pre.sum(axis=-1, keepdims=True), 1e-8)
    return pre.reshape(B, S, SPECTRAL_DIM)


def kernel(pos_3d, spectral_color, centers1, centers2, centers3,
           portal1_T, portal2_T, W_bands, b_bands, band_weights):
    from concourse.bass_utils import run_bass_kernel_spmd

    in_maps, equal_w = _build_inmaps(
        pos_3d, spectral_color, centers1, centers2, centers3,
        portal1_T, portal2_T, W_bands, b_bands, band_weights)
    nc = _get_compiled(equal_w)
    res = run_bass_kernel_spmd(nc, in_maps, core_ids=list(range(N_CORES)))
    return _unshard_out(res)


def run_traced(inputs, **kw):
    """Test-only: same launch as kernel() but trace=True."""
    from concourse.bass_utils import run_bass_kernel_spmd

    in_maps, equal_w = _build_inmaps(**inputs)
    nc = _get_compiled(equal_w)
    return run_bass_kernel_spmd(nc, in_maps, core_ids=list(range(N_CORES)),
                                trace=True, **kw)


if __name__ == "__main__":
    sys.path.insert(0, "/root/problem")
    import reference
    inputs = {k: np.asarray(v) for k, v in reference.setup_inputs().items()}
    out = kernel(**inputs)
    exp = np.asarray(reference.reference(**inputs))
    err = np.max(np.abs(out - exp)) / max(np.max(np.abs(exp)), 1e-12)
    print("Relative error:", err)


# revision 22
# speedup vs baseline: 1.1189x; 1.1189x over previous
"""Trainium2 Bass kernel for nn_InceptionTraversal (hierarchical sphere-softmax
MoE routing + per-band sigmoid routers).

Strategy (v3)
-------------
Host (numpy):
  * All distances d_s = |M_s p + u_s|^2 for the 84 spheres (4 + 16 + 64, with
    portal affines composed) are linear in psi = [x2,xy,xz,y2,yz,z2,x,y,z,1].
    Fold alpha = 1/(2T^2+eps) into Wd [10, 84].
  * Leaf-fold: d123[jlm] = d1[j] + d2[jl] + d3[jlm] is ALSO linear in psi ->
    extra 64 matmul columns, so the device needs no level-combine adds for
    the softmax numerators (Z1 cancels in the final normalize; raw d1 is
    never needed in the main phase).
  * Per-band routers are one block-diag matmul [64 -> 256] with sigmoid(x)
    = 0.5 + 0.5*tanh(x/2) folded into the weights.
  * Ship Phi = [psi(10); spectral(64)] bf16, pre-chunked [NSC, 74, CH].
Device (per core, 16384 tokens = 32 superchunks of 4x128-token groups):
  Phase A (sqrt table): bf16 matmul d-only [10 -> 84]; u = lam*sqrt(d+eps);
    DVE folds U123 = u1 + u2 + u3 into a [64/token] SBUF stash.
  Phase B (exp/tanh table; no barrier, ACT program order gates the switch):
    bf16 matmul [74 -> 400] = [router(256) | d23(80) | dfold(64)] per group.
    T3 = dfold + U123 added in-place in PSUM, then ONE fused exp over
    [d23|T3] gives E (denominator terms) and H (leaf numerators) in one
    ACT op; th = tanh(router).  Tail (Z3/Z2 reduces, R = 1/(Z2*Z3), m3e,
    band-sum, (sth+4)*m3e, row-normalize) is batched over superchunk PAIRS
    to halve DVE/GPSIMD instruction overhead; output written bf16.
Sharding: pure data-parallel over 8 cores (tokens split 8 ways).
"""

import sys

import numpy as np

if "/opt/trn_rl_repo" not in sys.path:
    sys.path.insert(0, "/opt/trn_rl_repo")

# ---- problem constants (hardcoded per contest contract) ----
N_DOM, N_SUB, N_CON = 4, 4, 4
SPECTRAL_DIM, N_BANDS = 64, 4
BAND_SIZE = SPECTRAL_DIM // N_BANDS
TEMP, LAM, EPS = 1.0, 0.1, 1e-8
ALPHA = 1.0 / (2.0 * TEMP * TEMP + EPS)
N_CORES = 8
B, S = 16, 8192
NTOK = B * S
TPC = NTOK // N_CORES          # tokens per core = 16384
GRP = 128                      # tokens per matmul group
G = 4                          # groups per superchunk (PSUM ping-pong)
CH = G * GRP                   # 512 tokens per superchunk
NSC = TPC // CH                # superchunks = 32
NS = 84                        # spheres (4 + 16 + 64)
NR = 256                       # router logits (64 leaves x 4 bands, (k,n))
NL = 64                        # leaves
KF = 74                        # Phi rows: 9 psi + 1 ones + 64 spectral
KD = 10                        # rows used by the distance matmul
# phase-B matmul column layout: [router 0:256 | d23 256:336 | dfold 336:400]
C_R0, C_D0, C_F0, NCOL = 0, NR, NR + 80, NR + 80 + NL

_compiled = {}


def _host_matrices(centers1, centers2, centers3, portal1_T, portal2_T,
                   W_bands, b_bands, band_weights):
    """Build Wd [10,84] (phase A), Wc [74,400] (phase B). float64 inside."""
    c1 = centers1.astype(np.float64)
    c2 = centers2.astype(np.float64)
    c3 = centers3.astype(np.float64)
    A1 = portal1_T[:, :, :3].astype(np.float64)
    b1 = portal1_T[:, :, 3].astype(np.float64)
    A2 = portal2_T[:, :, :3].astype(np.float64)
    b2 = portal2_T[:, :, 3].astype(np.float64)

    Ms = np.zeros((NS, 3, 3))
    us = np.zeros((NS, 3))
    s = 0
    for j in range(N_DOM):                     # level 1
        Ms[s] = np.eye(3)
        us[s] = -c1[j]
        s += 1
    for j in range(N_DOM):                     # level 2
        for l in range(N_SUB):
            Ms[s] = A1[j]
            us[s] = b1[j] - c2[j * N_SUB + l]
            s += 1
    for j in range(N_DOM):                     # level 3
        for l in range(N_SUB):
            jl = j * N_SUB + l
            M = A2[jl] @ A1[j]
            v = A2[jl] @ b1[j] + b2[jl]
            for m in range(N_CON):
                Ms[s] = M
                us[s] = v - c3[jl * N_CON + m]
                s += 1
    assert s == NS

    Wd = np.zeros((KD, NS))
    for i in range(NS):
        Q = Ms[i].T @ Ms[i]
        lin = 2.0 * (Ms[i].T @ us[i])
        Wd[:, i] = [Q[0, 0], 2 * Q[0, 1], 2 * Q[0, 2], Q[1, 1], 2 * Q[1, 2],
                    Q[2, 2], lin[0], lin[1], lin[2], us[i] @ us[i]]
    Wd *= ALPHA                                # PSUM d-cols = alpha * d_true

    w = np.exp(band_weights.astype(np.float64))
    w = w / w.sum()
    equal_w = bool(np.allclose(w, w[0], rtol=1e-6, atol=1e-9))

    Wc = np.zeros((KF, NCOL))
    # router cols: col k*4 + n = 0.5 * (x_band_n . W_bands[n,:,k] + b[n,k])
    Wr = np.zeros((SPECTRAL_DIM, SPECTRAL_DIM, N_BANDS))
    for n in range(N_BANDS):
        Wr[n * BAND_SIZE:(n + 1) * BAND_SIZE, :, n] = 0.5 * W_bands[n].astype(np.float64)
    Wc[KD:KF, C_R0:C_R0 + NR] = Wr.reshape(SPECTRAL_DIM, NR)
    Wc[KD - 1, C_R0:C_R0 + NR] = 0.5 * b_bands.astype(np.float64).T.reshape(NR)
    # raw d for levels 2+3 (E / Z denominators)
    Wc[0:KD, C_D0:C_D0 + 80] = Wd[:, 4:84]
    # leaf-folded d123 = d3 + d2[parent] + d1[grandparent]
    dfold = (Wd[:, 20:84]
             + np.repeat(Wd[:, 4:20], N_CON, axis=1)
             + np.repeat(Wd[:, 0:4], N_SUB * N_CON, axis=1))
    Wc[0:KD, C_F0:C_F0 + NL] = dfold
    return (Wd.astype(np.float32), Wc.astype(np.float32), equal_w,
            w.astype(np.float32))


def _host_phi(pos_3d, spectral_color):
    """Phi [74, NTOK] f32: rows [x2,xy,xz,y2,yz,z2,x,y,z,1, spectral...]."""
    p = pos_3d.reshape(-1, 3).astype(np.float32)
    x, y, z = p[:, 0], p[:, 1], p[:, 2]
    phi = np.empty((KF, NTOK), dtype=np.float32)
    phi[0] = x * x
    phi[1] = x * y
    phi[2] = x * z
    phi[3] = y * y
    phi[4] = y * z
    phi[5] = z * z
    phi[6] = x
    phi[7] = y
    phi[8] = z
    phi[9] = 1.0
    phi[KD:] = spectral_color.reshape(-1, SPECTRAL_DIM).astype(np.float32).T
    return np.ascontiguousarray(phi)


def _build_module(equal_w):
    import concourse.bacc as bacc
    import concourse.mybir as mybir
    import concourse.tile as tile

    f32 = mybir.dt.float32
    f16 = mybir.dt.float16
    bf16 = mybir.dt.bfloat16
    AF = mybir.ActivationFunctionType
    OP = mybir.AluOpType
    AX = mybir.AxisListType

    nc = bacc.Bacc("TRN2", target_bir_lowering=False)
    phi_d = nc.dram_tensor("phi", [NSC * KF, CH], f16, kind="ExternalInput")
    wd_d = nc.dram_tensor("wd", [KD, NS], f16, kind="ExternalInput")
    wc_d = nc.dram_tensor("wc", [KF, NCOL], f16, kind="ExternalInput")
    out_d = nc.dram_tensor("routing", [NSC * GRP, G * NL], f32,
                           kind="ExternalOutput")

    # numeric folds:  u = sqrt(sq_scale*dps + sq_bias) = lam*sqrt(d_true+eps)
    sq_scale = (LAM * LAM) / ALPHA
    sq_bias = LAM * LAM * EPS
    pre_add = 4.0 if equal_w else 1.0

    # activation() turns float biases into const APs - register ours.
    for cval in (sq_bias,):
        if (f32, cval) not in nc.const_aps.aps:
            ct = nc.alloc_sbuf_tensor(f"const-f32-{cval}", [128, 1], f32)
            nc.gpsimd.memset(ct.ap(), cval)
            nc.const_aps.aps[(f32, cval)] = ct.ap()
    nc.all_engine_barrier()

    NP = NSC // 2                      # superchunk pairs

    with tile.TileContext(nc) as tc:
        with (
            tc.tile_pool(name="const", bufs=1) as constp,
            tc.tile_pool(name="stash", bufs=1) as stashp,
            tc.tile_pool(name="io", bufs=3) as iop,
            tc.tile_pool(name="work", bufs=3) as wp,
            tc.tile_pool(name="tail", bufs=2) as tp,
            tc.tile_pool(name="ps", bufs=2, space="PSUM") as psp,
        ):
            wd_sb = constp.tile([KD, NS], f16)
            nc.sync.dma_start(wd_sb[:], wd_d[:])
            wc_sb = constp.tile([KF, NCOL], f16)
            nc.sync.dma_start(wc_sb[:], wc_d[:])
            if not equal_w:
                wt_sb = constp.tile([GRP, N_BANDS], f32)
                wt_dram = nc.dram_tensor("wt", [1, N_BANDS], f32,
                                         kind="ExternalInput")
                nc.sync.dma_start(wt_sb[:], wt_dram[:].partition_broadcast(GRP))

            u_stash = stashp.tile([GRP, NSC * G * NL], f16)

            # ---------------- Phase A: sqrt table set ----------------
            for pp in range(NP):
                u = wp.tile([GRP, 2, G, NS], f16, tag="u")
                for h in range(2):
                    sc = 2 * pp + h
                    psiA = iop.tile([KD, CH], f16, tag="psiA")
                    nc.sync.dma_start(psiA[:], phi_d[sc * KF:sc * KF + KD, :])
                    psA = psp.tile([GRP, G, 512], f32, tag="ps")
                    for g in range(G):
                        nc.tensor.matmul(
                            psA[:, g, 0:NS],
                            psiA[:, g * GRP:(g + 1) * GRP],
                            wd_sb[:],
                            start=True, stop=True,
                        )
                    nc.scalar.activation(u[:, h], psA[:, :, 0:NS],
                                         AF.Sqrt, bias=sq_bias, scale=sq_scale)
                U12 = wp.tile([GRP, 2, G, 16], f16, tag="U12")
                nc.vector.tensor_tensor(
                    U12.rearrange("p h g (j l) -> p h g j l", l=4),
                    u[:, :, :, 4:20].rearrange("p h g (j l) -> p h g j l", l=4),
                    u[:, :, :, 0:4].unsqueeze(4).broadcast_to((GRP, 2, G, 4, 4)),
                    OP.add)
                ust = u_stash[:, pp * (2 * G * NL):(pp + 1) * (2 * G * NL)]
                nc.vector.tensor_tensor(
                    ust.rearrange("p (h g jl m) -> p h g jl m", h=2, g=G, m=4),
                    u[:, :, :, 20:NS].rearrange("p h g (jl m) -> p h g jl m", m=4),
                    U12[:].unsqueeze(4).broadcast_to((GRP, 2, G, 16, 4)),
                    OP.add)

            # prefetch the first pair's phi chunks so the transfers run
            # during the barrier drain
            pf_tiles = []
            for sc in range(2):
                phiB = iop.tile([KF, CH], f16, tag="phiB")
                eng = nc.gpsimd if (sc % 2 == 0) else nc.sync
                eng.dma_start(phiB[:], phi_d[sc * KF:(sc + 1) * KF, :])
                pf_tiles.append(phiB)

            tc.strict_bb_all_engine_barrier()

            # ---------------- Phase B: exp/tanh table set ----------------
            # No engine barrier: the ACT sequencer runs all sqrts, then the
            # auto-inserted table switch, then the exps; other engines run
            # ahead freely.  Tail math is batched over superchunk pairs.
            for pp in range(NP):
                # X = exp(-U123): ray-decay leaf factor; depends only on the
                # phase-A stash, so emit it first to unblock the GP chain
                X = tp.tile([GRP, 2, G, NL], bf16, tag="X")
                ustp = u_stash[:, pp * (2 * G * NL):(pp + 1) * (2 * G * NL)]
                nc.scalar.activation(
                    X[:], ustp.rearrange("p (h g c) -> p h g c", h=2, g=G),
                    AF.Exp, scale=-1.0)

                EH = tp.tile([GRP, 2, G, 144], bf16, tag="EH")
                th = tp.tile([GRP, 2, G, NR], f16, tag="th")
                for h in range(2):
                    sc = 2 * pp + h
                    if pp == 0:
                        phiB = pf_tiles[h]
                    else:
                        phiB = iop.tile([KF, CH], f16, tag="phiB")
                        eng = nc.gpsimd if (sc % 2 == 0) else nc.sync
                        eng.dma_start(phiB[:], phi_d[sc * KF:(sc + 1) * KF, :])
                    psB = psp.tile([GRP, G, 512], f32, tag="ps")
                    for g in range(G):
                        nc.tensor.matmul(
                            psB[:, g, 0:NCOL],
                            phiB[:, g * GRP:(g + 1) * GRP],
                            wc_sb[:],
                            start=True, stop=True,
                        )
                    # one fused exp: E = exp(-d23) | Hf = exp(-dfold)
                    nc.scalar.activation(EH[:, h], psB[:, :, C_D0:C_D0 + 144],
                                         AF.Exp, scale=-1.0)
                    nc.scalar.activation(th[:, h], psB[:, :, C_R0:C_R0 + NR],
                                         AF.Tanh)

                E = EH[:, :, :, 0:80]
                Hf = EH[:, :, :, 80:144]
                # merged Z2|Z3 denominator sums: [p,h,g,20]
                Z23 = tp.tile([GRP, 2, G, 20], f32, tag="Z23")
                nc.vector.tensor_reduce(
                    Z23[:], E.rearrange("p h g (q m) -> p h g q m", m=4),
                    AX.X, OP.add)
                ZZ = tp.tile([GRP, 2, G, 16], f32, tag="ZZ")
                nc.vector.tensor_tensor(
                    ZZ.rearrange("p h g (j l) -> p h g j l", l=4),
                    Z23[:, :, :, 4:20].rearrange("p h g (j l) -> p h g j l", l=4),
                    Z23[:, :, :, 0:4].unsqueeze(4).broadcast_to((GRP, 2, G, 4, 4)),
                    OP.mult)
                R = tp.tile([GRP, 2, G, 16], f32, tag="R")
                nc.vector.reciprocal_approx_fast(
                    R[:].rearrange("p h g c -> p (h g c)"),
                    ZZ[:].rearrange("p h g c -> p (h g c)"))

                # H = Hf * X (leaf numerator incl ray decay), all-f16 on DVE
                HX = tp.tile([GRP, 2, G, NL], bf16, tag="HX")
                nc.gpsimd.tensor_tensor(HX[:], Hf, X[:], OP.mult)
                m3e = tp.tile([GRP, 2, G, NL], f32, tag="m3e")
                nc.vector.tensor_tensor(
                    m3e.rearrange("p h g (jl m) -> p h g jl m", m=4),
                    HX[:].rearrange("p h g (jl m) -> p h g jl m", m=4),
                    R[:].unsqueeze(4).broadcast_to((GRP, 2, G, 16, 4)),
                    OP.mult)

                # band sum of tanh in two f16 steps (2x DVE mode on step 1)
                thv = th[:].rearrange("p h g (k n) -> p h g k n", n=4)
                if equal_w:
                    s2 = tp.tile([GRP, 2, G, NL, 2], f16, tag="s2")
                    nc.vector.tensor_tensor(
                        s2[:], thv[:, :, :, :, 0:2], thv[:, :, :, :, 2:4],
                        OP.add)
                else:
                    thw = tp.tile([GRP, 2, G, NR], f16, tag="thw")
                    nc.vector.tensor_tensor(
                        thw.rearrange("p h g (k n) -> p h g k n", n=4), thv,
                        wt_sb[:].unsqueeze(1).unsqueeze(1).unsqueeze(1)
                        .broadcast_to((GRP, 2, G, NL, N_BANDS)),
                        OP.mult)
                    s2 = tp.tile([GRP, 2, G, NL, 2], f16, tag="s2")
                    thwv = thw.rearrange("p h g (k n) -> p h g k n", n=4)
                    nc.vector.tensor_tensor(
                        s2[:], thwv[:, :, :, :, 0:2], thwv[:, :, :, :, 2:4],
                        OP.add)
                sth = tp.tile([GRP, 2, G, NL], f32, tag="sth")
                nc.vector.tensor_tensor(
                    sth[:], s2[:, :, :, :, 0], s2[:, :, :, :, 1], OP.add)

                # unnormalized pre is stored; the host does the row-
                # normalize (single divide by the row sum)
                pre = tp.tile([GRP, 2, G, NL], f32, tag="pre")
                nc.vector.scalar_tensor_tensor(
                    pre[:], sth[:], pre_add, m3e[:], OP.add, OP.mult)

                nc.sync.dma_start(
                    out_d[pp * 2 * GRP:(pp + 1) * 2 * GRP, :].rearrange(
                        "(h p) (g k) -> p h g k", p=GRP, g=G),
                    pre[:])

    nc.finalize()
    return nc


def _get_compiled(equal_w):
    if equal_w not in _compiled:
        _compiled[equal_w] = _build_module(equal_w)
    return _compiled[equal_w]


def _chunk_rows(a, nrows):
    """[nrows, TPC] f32 -> [NSC*nrows, CH] fp16 (contiguous per superchunk)."""
    v = a.reshape(nrows, NSC, CH).transpose(1, 0, 2)
    return np.ascontiguousarray(v.reshape(NSC * nrows, CH)).astype(np.float16)


def _build_inmaps(pos_3d, spectral_color, centers1, centers2, centers3,
                  portal1_T, portal2_T, W_bands, b_bands, band_weights):
    Wd, Wc, equal_w, w_vec = _host_matrices(
        np.asarray(centers1), np.asarray(centers2), np.asarray(centers3),
        np.asarray(portal1_T), np.asarray(portal2_T),
        np.asarray(W_bands), np.asarray(b_bands), np.asarray(band_weights))
    phi = _host_phi(np.asarray(pos_3d), np.asarray(spectral_color))

    wd_b = Wd.astype(np.float16)
    wc_b = Wc.astype(np.float16)
    in_maps = []
    for c in range(N_CORES):
        pc = phi[:, c * TPC:(c + 1) * TPC]
        m = {
            "phi": _chunk_rows(pc, KF),
            "wd": wd_b,
            "wc": wc_b,
        }
        if not equal_w:
            m["wt"] = w_vec.reshape(1, N_BANDS)
        in_maps.append(m)
    return in_maps, equal_w


def _unshard_out(res):
    outs = []
    for c in range(N_CORES):
        o = np.asarray(res.results[c]["routing"]).astype(np.float32)
        o = o.reshape(NSC, GRP, G, NL).transpose(0, 2, 1, 3).reshape(TPC, NL)
        outs.append(o)
    pre = np.concatenate(outs, axis=0)
    # final row-normalize (matches reference's clip(sum, 1e-8))
    pre /= np.maximum(pre.sum(axis=-1, keepdims=True), 1e-8)
    return pre.reshape(B, S, SPECTRAL_DIM)


def kernel(pos_3d, spectral_color, centers1, centers2, centers3,
           portal1_T, portal2_T, W_bands, b_bands, band_weights):
    from concourse.bass_utils import run_bass_kernel_spmd

    in_maps, equal_w = _build_inmaps(
        pos_3d, spectral_color, centers1, centers2, centers3,
        portal1_T, portal2_T, W_bands, b_bands, band_weights)
    nc = _get_compiled(equal_w)
    res = run_bass_kernel_spmd(nc, in_maps, core_ids=list(range(N_CORES)))
    return _unshard_out(res)


def run_traced(inputs, **kw):
    """Test-only: same launch as kernel() but trace=True."""
    from concourse.bass_utils import run_bass_kernel_spmd

    in_maps, equal_w = _build_inmaps(**inputs)
    nc = _get_compiled(equal_w)
    return run_bass_kernel_spmd(nc, in_maps, core_ids=list(range(N_CORES)),
                                trace=True, **kw)


if __name__ == "__main__":
    sys.path.insert(0, "/root/problem")
    import reference
    inputs = {k: np.asarray(v) for k, v in reference.setup_inputs().items()}
    out = kernel(**inputs)
    exp = np.asarray(reference.reference(**inputs))
    err = np.max(np.abs(out - exp)) / max(np.max(np.abs(exp)), 1e-12)
    print("Relative error:", err)
